# revision 1
# baseline (speedup 1.0000x reference)
"""Trainium2 Bass kernel for nn_ConvolutionOneWay (equivariant GNN message passing).

Strategy (8 cores, edge-parallel by destination partition):
  - Receivers are split into 8 contiguous slices of 2500; core k owns slice k.
  - Edges are routed (host-side "all-to-all by destination partition") to the
    core owning their destination, sorted by local destination, grouped into
    128-receiver destination tiles, and padded with zero-attr edges (=> zero
    message) to a uniform chunk count so one static program serves all cores.
  - Node features are transformed once per core (lin1), stored node-major in
    DRAM; per-edge rows are fetched with batched indirect DMA (gather).
  - Per-edge tensor product runs on DVE/ACT with fused scalar_tensor_tensor
    ops, edge-major [128 edges x feat].
  - Segment-sum: one-hot selection matrix as the stationary matmul operand;
    PE accumulates rf [128 slots, 384] in PSUM across a dst tile's chunks,
    then PE transposes feed the receiver-side matmuls.
  - Receiver-side sc/lin2/lin3 + cos/sin gating finish each dst tile; each
    core writes its own [2500, 160] output slice. No collectives.

All DMA traffic is batched per destination tile (or larger) because each DMA
instruction costs ~0.6-1us of DGE overhead regardless of size.
All scalar normalization constants are folded into the weights on the host.
"""

import numpy as np

P = 128
DIM = 160
M_CORES = 8

_prog_cache = {}
_TRACE = False
_last_results = None


def _host_prep(inputs, n_cores, nrl, ns):
    f32 = np.float32
    s2 = lambda a: np.ascontiguousarray(a, dtype=f32)

    src = np.asarray(inputs["edge_src"])
    dst = np.asarray(inputs["edge_dst"])

    nt_d = (nrl + P - 1) // P  # dst tiles per core
    core_of = dst // nrl
    per_core = []
    max_tile_cnt = 1
    for k in range(n_cores):
        idx = np.nonzero(core_of == k)[0]
        ldst = dst[idx] - k * nrl
        order = np.argsort(ldst, kind="stable")
        idx = idx[order]
        ldst = ldst[order]
        tile_id = ldst // P
        counts = np.bincount(tile_id, minlength=nt_d)
        max_tile_cnt = max(max_tile_cnt, int(counts.max()))
        per_core.append((idx, ldst, tile_id, counts))
    ch = (max_tile_cnt + P - 1) // P  # 128-edge chunks per dst tile
    C = ch * P

    # ---- fold constants into weights ----
    fc_W1p = s2(inputs["fc_W1"]) / np.sqrt(16.0)
    W2 = s2(inputs["fc_W2"]) / np.sqrt(64.0)
    w_a, w_b, w_c, w_d = W2[:, :64], W2[:, 64:128], W2[:, 128:160], W2[:, 160:192]
    fc_W2p = np.concatenate([w_a, w_d / np.sqrt(3.0), w_b, w_c], axis=1)  # [64,192]
    inv = 1.0 / np.sqrt(16.0)  # NUM_NEIGHBORS
    fan = np.sqrt(96.0)
    W_conv = np.concatenate(
        [s2(inputs["W_lin2_0"]) * (inv / fan),
         0.1 * s2(inputs["W_lin3"]) * (inv / fan)], axis=1)  # [96,65]
    wpack = np.zeros((96, 545), f32)
    wpack[0:16, 0:64] = fc_W1p
    wpack[0:64, 64:256] = fc_W2p
    wpack[0:64, 256:320] = s2(inputs["W_lin1_0"]) / np.sqrt(64.0)
    wpack[0:32, 320:352] = s2(inputs["W_lin1_1"]) / np.sqrt(32.0)
    wpack[0:64, 352:416] = s2(inputs["W_sc0"]) / np.sqrt(64.0)
    wpack[0:32, 416:448] = s2(inputs["W_sc1"]) / np.sqrt(32.0)
    wpack[0:96, 448:513] = W_conv
    wpack[0:96, 513:545] = s2(inputs["W_lin2_1"]) * (inv / fan)

    # ---- sender features: transposed, i-major, padded ----
    ns_pad = ((ns + P - 1) // P) * P
    nt_n = ns_pad // P
    sx = s2(inputs["sender_input"])
    s0, s1 = sx[:, :64], sx[:, 64:].reshape(ns, 32, 3)
    sx0_T = np.zeros((64, ns_pad), f32)
    sx0_T[:, :ns] = s0.T
    sx1_T = np.zeros((32, 3, ns_pad), f32)
    sx1_T[:, :, :ns] = s1.transpose(1, 2, 0)
    sx_attr_t = np.zeros((P, nt_n), f32)
    sx_attr_t.reshape(-1)[:0] = 0  # keep contiguous
    a_pad = np.zeros((ns_pad,), f32)
    a_pad[:ns] = s2(inputs["sender_attr"]).ravel()
    sx_attr_t[:, :] = a_pad.reshape(nt_n, P).T

    shared = {"sx0_T": sx0_T, "sx1_T": sx1_T, "sx_attr_t": sx_attr_t,
              "wpack": wpack}

    rx = s2(inputs["receiver_input"])
    r_attr = s2(inputs["receiver_attr"])
    es_full = s2(inputs["edge_scalars"])
    ea_full = s2(inputs["edge_attr"])
    nrl_pad = nt_d * P

    maps = []
    for k in range(n_cores):
        idx, ldst, tile_id, counts = per_core[k]
        es_T = np.zeros((nt_d, 16, C), f32)
        epk = np.zeros((nt_d, P, ch, 5), f32)
        esrc = np.zeros((nt_d, P, ch), np.int32)
        pos = 0
        for t in range(nt_d):
            n = int(counts[t])
            e_ids = idx[pos:pos + n]
            j = np.arange(n)
            cc, pp = j // P, j % P
            es_T[t, :, :n] = es_full[e_ids].T
            epk[t, pp, cc, 0:4] = ea_full[e_ids]
            epk[t, pp, cc, 4] = (ldst[pos:pos + n] % P).astype(f32)
            esrc[t, pp, cc] = src[e_ids]
            pos += n
        rxs = rx[k * nrl:(k + 1) * nrl]
        r0, r1 = rxs[:, :64], rxs[:, 64:].reshape(nrl, 32, 3)
        rx0_T = np.zeros((64, nrl_pad), f32)
        rx0_T[:, :nrl] = r0.T
        rx1_T = np.zeros((32, 3, nrl_pad), f32)
        rx1_T[:, :, :nrl] = r1.transpose(1, 2, 0)
        ra_pad = np.zeros((nrl_pad,), f32)
        ra_pad[:nrl] = r_attr[k * nrl:(k + 1) * nrl].ravel()
        rx_attr_t = ra_pad.reshape(nt_d, P).T.copy()
        m = dict(shared)
        m.update({"es_T": es_T, "epk": epk, "esrc": esrc,
                  "rx0_T": rx0_T, "rx1_T": rx1_T, "rx_attr_t": rx_attr_t})
        maps.append(m)

    cfg = {"ns_pad": ns_pad, "nt_n": nt_n, "nrl": nrl,
           "nt_d": nt_d, "ch": ch, "C": C}
    return cfg, maps


def _build_program(cfg, n_cores, ablate=()):
    import concourse.bass as bass
    import concourse.bacc as bacc
    from concourse import mybir
    from concourse.tile import TileContext
    from concourse.masks import make_identity

    no_gather = "gather" in ablate
    no_tp = "tp" in ablate
    no_scatter = "scatter" in ablate
    no_mlp = "mlp" in ablate
    no_phasea = "phasea" in ablate

    f32 = mybir.dt.float32
    i32 = mybir.dt.int32
    AF = mybir.ActivationFunctionType
    OP = mybir.AluOpType
    PI_2 = float(np.pi / 2.0)

    ns_pad, nt_n = cfg["ns_pad"], cfg["nt_n"]
    nrl, nt_d, ch, C = cfg["nrl"], cfg["nt_d"], cfg["ch"], cfg["C"]
    GA = 4  # phase-A node tiles per DMA group

    nc = bacc.Bacc("TRN2", target_bir_lowering=False, debug=False,
                   num_devices=n_cores)

    sx0_T = nc.dram_tensor("sx0_T", [64, ns_pad], f32, kind="ExternalInput").ap()
    sx1_T = nc.dram_tensor("sx1_T", [32, 3, ns_pad], f32, kind="ExternalInput").ap()
    sx_attr_t = nc.dram_tensor("sx_attr_t", [P, nt_n], f32, kind="ExternalInput").ap()
    rx0_T = nc.dram_tensor("rx0_T", [64, nt_d * P], f32, kind="ExternalInput").ap()
    rx1_T = nc.dram_tensor("rx1_T", [32, 3, nt_d * P], f32, kind="ExternalInput").ap()
    rx_attr_t = nc.dram_tensor("rx_attr_t", [P, nt_d], f32, kind="ExternalInput").ap()
    es_T = nc.dram_tensor("es_T", [nt_d, 16, C], f32, kind="ExternalInput").ap()
    epk = nc.dram_tensor("epk", [nt_d, P, ch, 5], f32, kind="ExternalInput").ap()
    esrc = nc.dram_tensor("esrc", [nt_d, P, ch], i32, kind="ExternalInput").ap()
    wpack_d = nc.dram_tensor("wpack", [96, 545], f32, kind="ExternalInput").ap()
    out_d = nc.dram_tensor("out", [nrl, DIM], f32, kind="ExternalOutput").ap()
    f_dram = nc.dram_tensor("f_tab", [ns_pad, DIM], f32).ap()

    MUL, ADD, EQ = OP.mult, OP.add, OP.is_equal

    with TileContext(nc) as tc:
        with tc.tile_pool(name="wts", bufs=1) as wp, \
             tc.tile_pool(name="sb", bufs=3) as sb, \
             tc.tile_pool(name="big", bufs=2) as bigp, \
             tc.tile_pool(name="nsb", bufs=2) as nsb, \
             tc.tile_pool(name="hwp", bufs=2, space="PSUM") as hwp, \
             tc.tile_pool(name="rfp", bufs=2, space="PSUM") as rfp, \
             tc.tile_pool(name="tpp", bufs=2, space="PSUM") as tpp, \
             tc.tile_pool(name="ndp", bufs=2, space="PSUM") as ndp:

            # --- constants ---
            wt = wp.tile([96, 545], f32, name="wt")
            nc.sync.dma_start(out=wt[:], in_=wpack_d[:])
            W1 = wt[0:16, 0:64]
            W2 = wt[0:64, 64:256]
            Wl10 = wt[0:64, 256:320]
            Wl11 = wt[0:32, 320:352]
            Wsc0 = wt[0:64, 352:416]
            Wsc1 = wt[0:32, 416:448]
            Wconv = wt[0:96, 448:513]
            Wl21 = wt[0:96, 513:545]
            sat_all = wp.tile([P, nt_n], f32, name="sat_all")
            nc.sync.dma_start(out=sat_all[:], in_=sx_attr_t[:])
            rat_all = wp.tile([P, nt_d], f32, name="rat_all")
            nc.sync.dma_start(out=rat_all[:], in_=rx_attr_t[:])
            pi2 = wp.tile([P, 1], f32, name="pi2")
            nc.vector.memset(pi2[:], PI_2)
            iota_i = wp.tile([P, P], i32, name="iota_i")
            nc.gpsimd.iota(iota_i[:], pattern=[[1, P]], base=0, channel_multiplier=0)
            iota_f = wp.tile([P, P], f32, name="iota_f")
            nc.vector.tensor_copy(out=iota_f[:], in_=iota_i[:])
            ident = wp.tile([P, P], f32, name="ident")
            make_identity(nc, ident[:])
            wsb_const = None
            if no_mlp:
                wsb_const = wp.tile([P, 192], f32, name="wsb_const")
                nc.vector.memset(wsb_const[:], 0.5)

            # --- phase A: f table (lin1 over sender nodes) ---
            for g in ([] if no_phasea else range(0, nt_n, GA)):
                gt = min(GA, nt_n - g)
                cols = gt * P
                s0T = bigp.tile([64, GA * P], f32, tag="s0T")
                nc.sync.dma_start(out=s0T[:, :cols],
                                  in_=sx0_T[:, g * P:g * P + cols])
                s1T = bigp.tile([32, 3, GA * P], f32, tag="s1T")
                nc.sync.dma_start(out=s1T[:, :, :cols],
                                  in_=sx1_T[:, :, g * P:g * P + cols])
                fsb = bigp.tile([P, GA, DIM], f32, tag="fsb")
                for ti in range(gt):
                    nt = g + ti
                    cs = slice(ti * P, (ti + 1) * P)
                    fps = hwp.tile([P, DIM], f32, tag="hw")
                    nc.tensor.matmul(out=fps[:, 0:64], lhsT=s0T[:, cs], rhs=Wl10,
                                     start=True, stop=True)
                    for i in range(3):
                        nc.tensor.matmul(out=fps[:, 64 + 32 * i:96 + 32 * i],
                                         lhsT=s1T[:, i, cs], rhs=Wl11,
                                         start=True, stop=True)
                    nc.scalar.activation(out=fsb[:, ti, :], in_=fps[:], func=AF.Copy,
                                         scale=sat_all[:, nt:nt + 1])
                nc.sync.dma_start(
                    out=f_dram[g * P:g * P + cols, :].rearrange(
                        "(c p) f -> p c f", p=P),
                    in_=fsb[:, :gt, :])

            tc.strict_bb_all_engine_barrier()

            # --- phase B: edges, one dst tile at a time ---
            for dt in range(nt_d):
                est = bigp.tile([16, C], f32, tag="est")
                nc.sync.dma_start(out=est[:], in_=es_T[dt])
                ept = sb.tile([P, ch, 5], f32, tag="ept")
                nc.sync.dma_start(out=ept[:], in_=epk[dt])
                srt = sb.tile([P, ch], i32, tag="srt")
                nc.sync.dma_start(out=srt[:], in_=esrc[dt])
                xt = bigp.tile([P, ch, DIM], f32, tag="xt")
                if no_gather:
                    nc.gpsimd.memset(xt[:], 0.25)
                else:
                    for g in range(ch):
                        nc.gpsimd.indirect_dma_start(
                            out=xt[:, g, :], out_offset=None, in_=f_dram[:],
                            in_offset=bass.IndirectOffsetOnAxis(ap=srt[:, g:g + 1], axis=0))

                rf_ps = rfp.tile([P, 384], f32, tag="rf")
                for c in range(ch):
                    if no_mlp:
                        wsb = wsb_const
                    else:
                        hp = hwp.tile([64, P], f32, tag="hw")
                        nc.tensor.matmul(out=hp[:], lhsT=W1,
                                         rhs=est[:, c * P:(c + 1) * P],
                                         start=True, stop=True)
                        hs = sb.tile([64, P], f32, tag="hs")
                        nc.scalar.activation(out=hs[:], in_=hp[:], func=AF.Silu)
                        wps = hwp.tile([P, 192], f32, tag="hw")
                        nc.tensor.matmul(out=wps[:], lhsT=hs[:], rhs=W2,
                                         start=True, stop=True)
                        wsb = sb.tile([P, 192], f32, tag="wsb")
                        nc.vector.tensor_copy(out=wsb[:], in_=wps[:])

                    sel = sb.tile([P, P], f32, tag="sel")
                    mid = sb.tile([P, 384], f32, tag="mid")
                    if no_tp:
                        nc.vector.memset(mid[:], 0.1)
                        sel = iota_f
                    y0 = ept[:, c, 0:1]
                    if not no_tp:
                        nc.vector.tensor_scalar(out=sel[:], in0=iota_f[:],
                                            scalar1=ept[:, c, 4:5], scalar2=None,
                                            op0=EQ)
                    if not no_tp:
                        nc.vector.scalar_tensor_tensor(
                            out=mid[:, 0:64], in0=xt[:, c, 0:64], scalar=y0,
                            in1=wsb[:, 0:64], op0=MUL, op1=MUL)
                        td = sb.tile([P, 32], f32, tag="td")
                        td2 = sb.tile([P, 32], f32, tag="td2")
                        nc.scalar.activation(out=td[:], in_=xt[:, c, 64:96],
                                             func=AF.Copy, scale=ept[:, c, 1:2])
                        nc.vector.scalar_tensor_tensor(
                            out=td2[:], in0=xt[:, c, 96:128], scalar=ept[:, c, 2:3],
                            in1=td[:], op0=MUL, op1=ADD)
                        nc.vector.scalar_tensor_tensor(
                            out=td[:], in0=xt[:, c, 128:160], scalar=ept[:, c, 3:4],
                            in1=td2[:], op0=MUL, op1=ADD)
                        nc.vector.tensor_tensor(out=mid[:, 64:96], in0=td[:],
                                                in1=wsb[:, 64:96], op=MUL)
                        for i in range(3):
                            nc.vector.scalar_tensor_tensor(
                                out=mid[:, 96 + 96 * i:160 + 96 * i],
                                in0=xt[:, c, 0:64], scalar=ept[:, c, 1 + i:2 + i],
                                in1=wsb[:, 96:160], op0=MUL, op1=MUL)
                            nc.vector.scalar_tensor_tensor(
                                out=mid[:, 160 + 96 * i:192 + 96 * i],
                                in0=xt[:, c, 64 + 32 * i:96 + 32 * i], scalar=y0,
                                in1=wsb[:, 160:192], op0=MUL, op1=MUL)
                    if not (no_scatter and c > 0):
                        nc.tensor.matmul(out=rf_ps[:], lhsT=sel[:], rhs=mid[:],
                                         start=(c == 0),
                                         stop=(c == ch - 1) or no_scatter,
                                         skip_group_check=no_scatter)

                # --- finalize dst tile ---
                rf_sb = nsb.tile([P, 384], f32, tag="rfsb")
                nc.scalar.activation(out=rf_sb[:], in_=rf_ps[:], func=AF.Copy)
                rsb = []
                for j in range(4):
                    tp_ps = tpp.tile([96, P], f32, tag="tp", name=f"tp{j}")
                    nc.tensor.transpose(out=tp_ps[:], in_=rf_sb[:, 96 * j:96 * j + 96],
                                        identity=ident[:])
                    t = nsb.tile([96, P], f32, tag=f"rsb{j}", name=f"rsb{j}")
                    nc.scalar.activation(out=t[:], in_=tp_ps[:], func=AF.Copy)
                    rsb.append(t)
                ds = slice(dt * P, (dt + 1) * P)
                r0T = nsb.tile([64, P], f32, tag="r0T")
                nc.sync.dma_start(out=r0T[:], in_=rx0_T[:, ds])
                r1T = nsb.tile([32, 3, P], f32, tag="r1T")
                nc.sync.dma_start(out=r1T[:], in_=rx1_T[:, :, ds])
                rat = rat_all[:, dt:dt + 1]

                nps = ndp.tile([P, 321], f32, tag="ndp")
                nc.tensor.matmul(out=nps[:, 0:64], lhsT=r0T[:], rhs=Wsc0,
                                 start=True, stop=True)
                for i in range(3):
                    nc.tensor.matmul(out=nps[:, 64 + 32 * i:96 + 32 * i],
                                     lhsT=r1T[:, i, :], rhs=Wsc1,
                                     start=True, stop=True)
                nc.tensor.matmul(out=nps[:, 160:225], lhsT=rsb[0][:], rhs=Wconv,
                                 start=True, stop=True)
                for i in range(3):
                    nc.tensor.matmul(out=nps[:, 225 + 32 * i:257 + 32 * i],
                                     lhsT=rsb[1 + i][:], rhs=Wl21,
                                     start=True, stop=True)

                ang = nsb.tile([P, 1], f32, tag="ang")
                nc.vector.tensor_scalar(out=ang[:], in0=nps[:, 224:225],
                                        scalar1=rat, scalar2=None, op0=MUL)
                cst = nsb.tile([P, 2], f32, tag="cst")
                nc.scalar.activation(out=cst[:, 0:1], in_=ang[:], func=AF.Sin,
                                     bias=pi2[:, 0:1])
                nc.scalar.activation(out=cst[:, 1:2], in_=ang[:], func=AF.Sin)
                scs = nsb.tile([P, DIM], f32, tag="scs")
                nc.vector.tensor_scalar(out=scs[:], in0=nps[:, 0:160],
                                        scalar1=rat, scalar2=None, op0=MUL)
                cvs = nsb.tile([P, DIM], f32, tag="cvs")
                nc.vector.tensor_scalar(out=cvs[:, 0:64], in0=nps[:, 160:224],
                                        scalar1=rat, scalar2=None, op0=MUL)
                nc.vector.tensor_scalar(out=cvs[:, 64:160], in0=nps[:, 225:321],
                                        scalar1=rat, scalar2=None, op0=MUL)
                tmp = nsb.tile([P, DIM], f32, tag="tmp")
                nc.vector.tensor_scalar(out=tmp[:], in0=scs[:],
                                        scalar1=cst[:, 0:1], scalar2=None, op0=MUL)
                outt = nsb.tile([P, DIM], f32, tag="outt")
                nc.vector.scalar_tensor_tensor(
                    out=outt[:, 0:64], in0=cvs[:, 0:64], scalar=cst[:, 1:2],
                    in1=tmp[:, 0:64], op0=MUL, op1=ADD)
                for i in range(3):
                    nc.vector.scalar_tensor_tensor(
                        out=outt[:, 64 + i:160:3],
                        in0=cvs[:, 64 + 32 * i:96 + 32 * i], scalar=cst[:, 1:2],
                        in1=tmp[:, 64 + 32 * i:96 + 32 * i], op0=MUL, op1=ADD)
                rows = min(P, nrl - dt * P)
                nc.sync.dma_start(out=out_d[dt * P:dt * P + rows, :],
                                  in_=outt[:rows, :])
    nc.compile()
    return nc


def _run(inputs, n_cores, nrl, ns, nr):
    from concourse.bass_utils import run_bass_kernel_spmd

    cfg, maps = _host_prep(inputs, n_cores, nrl, ns)
    key = tuple(sorted(cfg.items()))
    if key not in _prog_cache:
        _prog_cache[key] = _build_program(cfg, n_cores)
    nc = _prog_cache[key]
    res = run_bass_kernel_spmd(nc, maps, list(range(n_cores)), trace=_TRACE)
    global _last_results
    _last_results = res
    out = np.concatenate([res.results[k]["out"] for k in range(n_cores)], axis=0)
    return out[:nr]


def kernel(**inputs):
    ns = inputs["sender_input"].shape[0]
    nr = inputs["receiver_input"].shape[0]
    nrl = nr // M_CORES
    return _run(inputs, M_CORES, nrl, ns, nr)

